# revision 16
# baseline (speedup 1.0000x reference)
"""Trainium2 Bass kernel for CausalDownsamplingLRU.

Algorithm (per core = one batch element; 8 cores, data-parallel over batch):
  With lam = r*e^{i theta} (per state n), h_t = lam*h_{t-1} + Bu_t, and only
  y[:, -DS:] needed:

  1. Input GEMMs (fp16, PE): Bu^T[n,t] = (gamma*B)^T.T @ x^T  (re & im planes)
  2. FIRST half (t < 1024): only h_{1023} is needed (the carry into the
     output window). Computed as a weighted reduction
        h_1023 = sum_s lam^{1023-s} Bu_s
     via tensor_tensor_reduce (elementwise mult + free-dim accumulate),
     4 TTR ops per n-block with chained accumulator init.
  3. SECOND half: phase twist e_j = e^{-i j theta} (.) Bu_{1024+j} decouples
     the complex recurrence into two REAL per-partition scans
        s_j = r*s_{j-1} + e_j   (tensor_tensor_scan, fp32 state),
     with initial s_{-1} = e^{i theta} * h_1023.
  4. Untwist h = e^{+i j theta} (.) s, then output GEMMs:
        y^T = C_re^T.T @ h_re + (-C_im^T).T @ h_im + D^T.T @ x^T
"""
import numpy as np

import concourse.bass as bass
import concourse.bacc as bacc
import concourse.mybir as mybir
from concourse.tile import TileContext
from concourse.bass_utils import run_bass_kernel_spmd

BATCH, T, IN, OUT, N = 8, 2048, 512, 512, 512
DS = 1024
P = 128
NB = N // P    # 4 state blocks
IBN = IN // P  # 4 input blocks
OBN = OUT // P # 4 output blocks
HF = 1024      # half length (= DS)
HH = 512       # matmul moving free-dim (PSUM bank limit for f32 out)

f32 = mybir.dt.float32
f16 = mybir.dt.float16
AOP = mybir.AluOpType

GP_TT = False   # offload untwist partner products to gpsimd

_CACHE = {}


def _build_nc():
    if "nc" in _CACHE:
        return _CACHE["nc"]
    nc = bacc.Bacc()
    xT = nc.dram_tensor("xT", [IN, HF], f16, kind="ExternalInput")  # second half only
    xh = nc.dram_tensor("xh", [HF, IN], f16, kind="ExternalInput")  # first half, untransposed
    btr = nc.dram_tensor("btr", [IN, N], f16, kind="ExternalInput")
    bti = nc.dram_tensor("bti", [IN, N], f16, kind="ExternalInput")
    ctr = nc.dram_tensor("ctr", [N, OUT], f16, kind="ExternalInput")
    ctin = nc.dram_tensor("ctin", [N, OUT], f16, kind="ExternalInput")
    dtw = nc.dram_tensor("dtw", [IN, OUT], f16, kind="ExternalInput")
    vreT = nc.dram_tensor("vreT", [HF, N], f16, kind="ExternalInput")
    vimT = nc.dram_tensor("vimT", [HF, N], f16, kind="ExternalInput")
    bnr = nc.dram_tensor("bnr", [N, IN], f16, kind="ExternalInput")
    bni = nc.dram_tensor("bni", [N, IN], f16, kind="ExternalInput")
    cosj = nc.dram_tensor("cosj", [N, HF], f16, kind="ExternalInput")
    sinj = nc.dram_tensor("sinj", [N, HF], f16, kind="ExternalInput")
    rb = nc.dram_tensor("rb", [N, HF], f32, kind="ExternalInput")
    # rot columns: 0=cos(theta), 1=-sin(theta), 2=sin(theta)
    rot = nc.dram_tensor("rot", [N, 3], f32, kind="ExternalInput")
    yT = nc.dram_tensor("yT", [OUT, DS], f32, kind="ExternalOutput")

    with TileContext(nc) as tc:
        with (
            tc.tile_pool(name="const", bufs=1) as cp,
            tc.tile_pool(name="xt", bufs=4) as xp,
            tc.tile_pool(name="bups", bufs=3, space="PSUM") as bp,
            tc.tile_pool(name="bu", bufs=2) as up,
            tc.tile_pool(name="tw", bufs=2) as wp,
            tc.tile_pool(name="hh", bufs=1) as hp,
            tc.tile_pool(name="carry", bufs=1) as kp,
            tc.tile_pool(name="yps", bufs=5, space="PSUM") as yp,
            tc.tile_pool(name="ysb", bufs=2) as op_,
        ):
            def load_const(dram, rows, dtype, tagp):
                tiles = []
                for i in range(rows // P):
                    t = cp.tile([P, dram.shape[1]], dtype, tag=f"{tagp}{i}",
                                name=f"{tagp}{i}")
                    nc.sync.dma_start(t[:], dram[i * P:(i + 1) * P, :])
                    tiles.append(t)
                return tiles

            # ---- x + input weights first so PE can start ASAP ----
            vreT_t = load_const(vreT, HF, f16, "vreT")
            vimT_t = load_const(vimT, HF, f16, "vimT")
            xh_t = load_const(xh, HF, f16, "xh")
            bnr_t = load_const(bnr, N, f16, "bnr")
            bni_t = load_const(bni, N, f16, "bni")
            btr_t = load_const(btr, IN, f16, "btr")
            xts = []
            for ib in range(IBN):
                xt_t = xp.tile([P, HF], f16, tag="xt", name="xt")
                nc.sync.dma_start(xt_t[:], xT[ib * P:(ib + 1) * P, :])
                xts.append(xt_t)
            bti_t = load_const(bti, IN, f16, "bti")
            cos_t = load_const(cosj, N, f16, "cos")
            sin_t = load_const(sinj, N, f16, "sin")
            rb_t = load_const(rb, N, f32, "rb")
            rot_t = load_const(rot, N, f32, "rot")

            def input_gemm(nb, pl_tiles, name):
                """Bu^T[n-block, second half] -> fp16 SBUF tile [P, HF]."""
                nsl = slice(nb * P, (nb + 1) * P)
                bu_t = up.tile([P, HF], f16, tag=name, name=name)
                for h in range(HF // HH):
                    t0 = h * HH
                    ps = bp.tile([P, HH], f32, tag="bups", name="bups")
                    for ib in range(IBN):
                        nc.tensor.matmul(
                            ps[:],
                            pl_tiles[ib][:, nsl],
                            xts[ib][:, t0:t0 + HH],
                            start=(ib == 0),
                            stop=(ib == IBN - 1),
                        )
                    nc.scalar.copy(bu_t[:, h * HH:(h + 1) * HH], ps[:])
                return bu_t

            def w_gemm(nb, v_tiles, name):
                """W[n-block, i] = sum_s V[n,s] x[s,i] -> fp16 tile [P, IN]."""
                nsl = slice(nb * P, (nb + 1) * P)
                ps = bp.tile([P, IN], f32, tag="bups", name="bups")
                for sb in range(HF // P):
                    nc.tensor.matmul(
                        ps[:],
                        v_tiles[sb][:, nsl],
                        xh_t[sb][:],
                        start=(sb == 0),
                        stop=(sb == HF // P - 1),
                    )
                w_t = up.tile([P, IN], f16, tag=name, name=name)
                nc.scalar.copy(w_t[:], ps[:])
                return w_t

            # ---- first half: W = V @ x_half (PE), then h_1023 = sum_i B~.W ----
            acc_re, acc_im = [], []
            for nb in range(NB):
                w_re = w_gemm(nb, vreT_t, "wre")
                w_im = w_gemm(nb, vimT_t, "wim")
                dump = wp.tile([P, IN], f16, tag="dump", name="dump")
                s1 = kp.tile([P, 1], f32, tag=f"s1{nb}", name=f"s1{nb}")
                s2 = kp.tile([P, 1], f32, tag=f"s2{nb}", name=f"s2{nb}")
                s3 = kp.tile([P, 1], f32, tag=f"s3{nb}", name=f"s3{nb}")
                s4 = kp.tile([P, 1], f32, tag=f"s4{nb}", name=f"s4{nb}")
                a_re = kp.tile([P, 1], f32, tag=f"are{nb}", name=f"are{nb}")
                a_im = kp.tile([P, 1], f32, tag=f"aim{nb}", name=f"aim{nb}")
                # h1023_re = sum_i (Bre.Wre - Bim.Wim); im = sum_i (Bre.Wim + Bim.Wre)
                nc.vector.scalar_tensor_tensor(
                    dump[:], bnr_t[nb][:], 1.0, w_re[:], AOP.bypass, AOP.mult,
                    accum_out=s1[:])
                nc.vector.scalar_tensor_tensor(
                    dump[:], bni_t[nb][:], 1.0, w_im[:], AOP.bypass, AOP.mult,
                    accum_out=s2[:])
                nc.vector.scalar_tensor_tensor(
                    dump[:], bnr_t[nb][:], 1.0, w_im[:], AOP.bypass, AOP.mult,
                    accum_out=s3[:])
                nc.vector.scalar_tensor_tensor(
                    dump[:], bni_t[nb][:], 1.0, w_re[:], AOP.bypass, AOP.mult,
                    accum_out=s4[:])
                nc.vector.tensor_tensor(a_re[:], s1[:], s2[:], AOP.subtract)
                nc.vector.tensor_tensor(a_im[:], s3[:], s4[:], AOP.add)
                acc_re.append(a_re)
                acc_im.append(a_im)

            # ---- second half: GEMMs + twist + scans + untwist ----
            hh_re, hh_im = [], []
            for nb in range(NB):
                buB_r = input_gemm(nb, btr_t, "buBr")
                buB_i = input_gemm(nb, bti_t, "buBi")

                # carry rotation: init = e^{i theta} * h_1023
                i_re = kp.tile([P, 1], f32, tag=f"ire{nb}", name=f"ire{nb}")
                i_im = kp.tile([P, 1], f32, tag=f"iim{nb}", name=f"iim{nb}")
                u_re = kp.tile([P, 1], f32, tag=f"ure{nb}", name=f"ure{nb}")
                u_im = kp.tile([P, 1], f32, tag=f"uim{nb}", name=f"uim{nb}")
                nc.scalar.mul(u_re[:], acc_re[nb][:], rot_t[nb][:, 0:1])
                nc.vector.scalar_tensor_tensor(
                    i_re[:], acc_im[nb][:], rot_t[nb][:, 1:2], u_re[:],
                    AOP.mult, AOP.add)
                nc.scalar.mul(u_im[:], acc_im[nb][:], rot_t[nb][:, 0:1])
                nc.vector.scalar_tensor_tensor(
                    i_im[:], acc_re[nb][:], rot_t[nb][:, 2:3], u_im[:],
                    AOP.mult, AOP.add)

                # twist: e = e^{-i j theta} * Bu
                p1 = wp.tile([P, HF], f16, tag="p1", name="p1")
                p2 = wp.tile([P, HF], f16, tag="p2", name="p2")
                p3 = wp.tile([P, HF], f16, tag="p3", name="p3")
                p4 = wp.tile([P, HF], f16, tag="p4", name="p4")
                e_re = wp.tile([P, HF], f16, tag="ere", name="ere")
                e_im = wp.tile([P, HF], f16, tag="eim", name="eim")
                nc.vector.tensor_tensor(p1[:], cos_t[nb][:], buB_r[:], AOP.mult)
                nc.vector.tensor_tensor(p2[:], sin_t[nb][:], buB_i[:], AOP.mult)
                nc.vector.tensor_tensor(e_re[:], p1[:], p2[:], AOP.add)
                nc.vector.tensor_tensor(p3[:], cos_t[nb][:], buB_i[:], AOP.mult)
                nc.vector.tensor_tensor(p4[:], sin_t[nb][:], buB_r[:], AOP.mult)
                nc.vector.tensor_tensor(e_im[:], p3[:], p4[:], AOP.subtract)

                # real scans (fp32 state), split in halves for pipelining
                h_re = wp.tile([P, HF], f16, tag="hre", name="hre")
                h_im = wp.tile([P, HF], f16, tag="him", name="him")
                q1 = wp.tile([P, HF], f16, tag="q1", name="q1")
                q2 = wp.tile([P, HF], f16, tag="q2", name="q2")
                q3 = wp.tile([P, HF], f16, tag="q3", name="q3")
                q4 = wp.tile([P, HF], f16, tag="q4", name="q4")
                hhr = hp.tile([P, HF], f16, tag=f"hhr{nb}", name=f"hhr{nb}")
                hhi = hp.tile([P, HF], f16, tag=f"hhi{nb}", name=f"hhi{nb}")
                for h in range(HF // HH):
                    hs = slice(h * HH, (h + 1) * HH)
                    ir = i_re[:, 0:1] if h == 0 else h_re[:, h * HH - 1:h * HH]
                    ii = i_im[:, 0:1] if h == 0 else h_im[:, h * HH - 1:h * HH]
                    nc.vector.tensor_tensor_scan(
                        h_re[:, hs], rb_t[nb][:, hs], e_re[:, hs], ir,
                        AOP.mult, AOP.add)
                    nc.vector.tensor_tensor_scan(
                        h_im[:, hs], rb_t[nb][:, hs], e_im[:, hs], ii,
                        AOP.mult, AOP.add)
                    # untwist: hh = e^{+i j theta} * h
                    nc.vector.tensor_tensor(q1[:, hs], cos_t[nb][:, hs], h_re[:, hs], AOP.mult)
                    nc.vector.tensor_tensor(q2[:, hs], sin_t[nb][:, hs], h_im[:, hs], AOP.mult)
                    nc.vector.tensor_tensor(hhr[:, hs], q1[:, hs], q2[:, hs], AOP.subtract)
                    nc.vector.tensor_tensor(q3[:, hs], cos_t[nb][:, hs], h_im[:, hs], AOP.mult)
                    nc.vector.tensor_tensor(q4[:, hs], sin_t[nb][:, hs], h_re[:, hs], AOP.mult)
                    nc.vector.tensor_tensor(hhi[:, hs], q3[:, hs], q4[:, hs], AOP.add)
                hh_re.append(hhr)
                hh_im.append(hhi)

            # ---- output weights (queue-ordered after input-side DMAs) ----
            ctr_t = load_const(ctr, N, f16, "ctr")
            ctin_t = load_const(ctin, N, f16, "ctin")
            dtw_t = load_const(dtw, IN, f16, "dtw")

            # ---- output GEMMs ----
            for h in range(DS // HH):
                hsl = slice(h * HH, (h + 1) * HH)
                xsl = slice(h * HH, (h + 1) * HH)
                for ob in range(OBN):
                    osl = slice(ob * P, (ob + 1) * P)
                    ps = yp.tile([P, HH], f32, tag="yps", name="yps")
                    nmm = 2 * NB + IBN
                    ops = []
                    for nb in range(NB - 1):
                        ops.append((ctr_t[nb][:, osl], hh_re[nb][:, hsl]))
                        ops.append((ctin_t[nb][:, osl], hh_im[nb][:, hsl]))
                    for ib in range(IBN):
                        ops.append((dtw_t[ib][:, osl], xts[ib][:, xsl]))
                    ops.append((ctr_t[NB - 1][:, osl], hh_re[NB - 1][:, hsl]))
                    ops.append((ctin_t[NB - 1][:, osl], hh_im[NB - 1][:, hsl]))
                    for k, (w, m) in enumerate(ops):
                        nc.tensor.matmul(ps[:], w, m,
                                         start=(k == 0), stop=(k == nmm - 1))
                    ysb = op_.tile([P, HH], f32, tag="ysb", name="ysb")
                    nc.scalar.copy(ysb[:], ps[:])
                    nc.sync.dma_start(yT[osl, hsl], ysb[:])

    nc.compile()
    nc.finalize()
    _CACHE["nc"] = nc
    return nc


def _legalize_waits(nc):
    """This toolchain's walrus accepts only ONE sync-wait per instruction
    (NEURON_ISA_TPB_EVENTS has a single wait slot); Tile's scheduler can emit
    several. Splice wait-carrying NoOps immediately before each offender —
    semantically identical blocking point, one wait per instruction."""
    cnt = 0
    for f in nc.m.functions:
        for bb in f.blocks:
            out = []
            changed = False
            for ins in bb.instructions:
                si = ins.sync_info
                waits = list(si.on_wait) if si and si.on_wait else []
                if len(waits) > 1:
                    changed = True
                    for w in waits[:-1]:
                        nop = mybir.InstNoOp(name=f"waitnop-{cnt}")
                        cnt += 1
                        nop.engine = ins.engine
                        nop.sync_info = mybir.SyncInfo(on_wait=[w], on_update=[])
                        nc.register_instruction(nop)
                        out.append(nop)
                    ins.sync_info = mybir.SyncInfo(
                        on_wait=[waits[-1]], on_update=list(si.on_update or []))
                out.append(ins)
            if changed:
                bb.instructions = out


def _host_prep(x, nu_log, theta_log, gamma_log, B_re, B_im, C_re, C_im, D):
    f64 = np.float64
    nu = np.asarray(nu_log, f64)
    th = np.asarray(theta_log, f64)
    gl = np.asarray(gamma_log, f64)
    r = np.exp(-np.exp(nu))
    theta = np.exp(th)
    gamma = np.exp(gl)

    shared = {
        "btr": np.ascontiguousarray((gamma[:, None] * np.asarray(B_re, f64)).T).astype(np.float16),
        "bti": np.ascontiguousarray((gamma[:, None] * np.asarray(B_im, f64)).T).astype(np.float16),
        "ctr": np.ascontiguousarray(np.asarray(C_re, f64).T).astype(np.float16),
        "ctin": np.ascontiguousarray((-np.asarray(C_im, f64)).T).astype(np.float16),
        "dtw": np.ascontiguousarray(np.asarray(D, f64).T).astype(np.float16),
    }
    j = np.arange(HF, dtype=f64)
    ang = theta[:, None] * j[None, :]
    shared["cosj"] = np.cos(ang).astype(np.float16)
    shared["sinj"] = np.sin(ang).astype(np.float16)
    # V = lam^{1023-s} = r^{1023-s} e^{i (1023-s) theta}, shipped transposed [s, n]
    e = (HF - 1) - j
    mag = np.exp(np.log(r)[:, None] * e[None, :])
    angv = theta[:, None] * e[None, :]
    shared["vreT"] = np.ascontiguousarray((mag * np.cos(angv)).T).astype(np.float16)
    shared["vimT"] = np.ascontiguousarray((mag * np.sin(angv)).T).astype(np.float16)
    shared["bnr"] = (gamma[:, None] * np.asarray(B_re, f64)).astype(np.float16)
    shared["bni"] = (gamma[:, None] * np.asarray(B_im, f64)).astype(np.float16)
    shared["rb"] = np.ascontiguousarray(
        np.broadcast_to(r[:, None].astype(np.float32), (N, HF)))
    shared["rot"] = np.stack(
        [np.cos(theta), -np.sin(theta), np.sin(theta)], axis=1).astype(np.float32)

    x = np.asarray(x, np.float32)
    in_maps = []
    for b in range(BATCH):
        m = dict(shared)
        m["xT"] = np.ascontiguousarray(x[b, HF:].T).astype(np.float16)
        m["xh"] = np.ascontiguousarray(x[b, :HF]).astype(np.float16)
        in_maps.append(m)
    return in_maps


def _run(in_maps, trace=False):
    nc = _build_nc()
    return run_bass_kernel_spmd(nc, in_maps, core_ids=list(range(BATCH)), trace=trace)


def kernel(**inputs):
    in_maps = _host_prep(**inputs)
    res = _run(in_maps, trace=False)
    y = np.stack([np.ascontiguousarray(res.results[b]["yT"].T) for b in range(BATCH)])
    return y.astype(np.float32)


def kernel_traced(**inputs):
    """Like kernel() but returns (y, exec_time_ns). Used by test.py."""
    in_maps = _host_prep(**inputs)
    res = _run(in_maps, trace=True)
    y = np.stack([np.ascontiguousarray(res.results[b]["yT"].T) for b in range(BATCH)])
    return y.astype(np.float32), res.exec_time_ns


# revision 18
# speedup vs baseline: 1.0042x; 1.0042x over previous
"""Trainium2 Bass kernel for CausalDownsamplingLRU.

Algorithm (per core = one batch element; 8 cores, data-parallel over batch):
  With lam = r*e^{i theta} (per state n), h_t = lam*h_{t-1} + Bu_t, and only
  y[:, -DS:] needed:

  1. Input GEMMs (fp16, PE): Bu^T[n,t] = (gamma*B)^T.T @ x^T  (re & im planes)
  2. FIRST half (t < 1024): only h_{1023} is needed (the carry into the
     output window). Computed as a weighted reduction
        h_1023 = sum_s lam^{1023-s} Bu_s
     via tensor_tensor_reduce (elementwise mult + free-dim accumulate),
     4 TTR ops per n-block with chained accumulator init.
  3. SECOND half: phase twist e_j = e^{-i j theta} (.) Bu_{1024+j} decouples
     the complex recurrence into two REAL per-partition scans
        s_j = r*s_{j-1} + e_j   (tensor_tensor_scan, fp32 state),
     with initial s_{-1} = e^{i theta} * h_1023.
  4. Untwist h = e^{+i j theta} (.) s, then output GEMMs:
        y^T = C_re^T.T @ h_re + (-C_im^T).T @ h_im + D^T.T @ x^T
"""
import numpy as np

import concourse.bass as bass
import concourse.bacc as bacc
import concourse.mybir as mybir
from concourse.tile import TileContext
from concourse.bass_utils import run_bass_kernel_spmd

BATCH, T, IN, OUT, N = 8, 2048, 512, 512, 512
DS = 1024
P = 128
NB = N // P    # 4 state blocks
IBN = IN // P  # 4 input blocks
OBN = OUT // P # 4 output blocks
HF = 1024      # half length (= DS)
HH = 512       # matmul moving free-dim (PSUM bank limit for f32 out)

f32 = mybir.dt.float32
f16 = mybir.dt.float16
AOP = mybir.AluOpType

GP_TT = False   # offload untwist partner products to gpsimd

_CACHE = {}


def _build_nc():
    if "nc" in _CACHE:
        return _CACHE["nc"]
    nc = bacc.Bacc()
    xT = nc.dram_tensor("xT", [IN, HF], f16, kind="ExternalInput")  # second half only
    xh = nc.dram_tensor("xh", [HF, IN], f16, kind="ExternalInput")  # first half, untransposed
    btr = nc.dram_tensor("btr", [IN, N], f16, kind="ExternalInput")
    bti = nc.dram_tensor("bti", [IN, N], f16, kind="ExternalInput")
    ctr = nc.dram_tensor("ctr", [N, OUT], f16, kind="ExternalInput")
    ctin = nc.dram_tensor("ctin", [N, OUT], f16, kind="ExternalInput")
    dtw = nc.dram_tensor("dtw", [IN, OUT], f16, kind="ExternalInput")
    vreT = nc.dram_tensor("vreT", [HF, N], f16, kind="ExternalInput")
    vimT = nc.dram_tensor("vimT", [HF, N], f16, kind="ExternalInput")
    bnr = nc.dram_tensor("bnr", [N, IN], f16, kind="ExternalInput")
    bni = nc.dram_tensor("bni", [N, IN], f16, kind="ExternalInput")
    cosj = nc.dram_tensor("cosj", [N, HF], f16, kind="ExternalInput")
    sinj = nc.dram_tensor("sinj", [N, HF], f16, kind="ExternalInput")
    rb = nc.dram_tensor("rb", [N, 1], f32, kind="ExternalInput")
    # rot columns: 0=cos(theta), 1=-sin(theta), 2=sin(theta)
    rot = nc.dram_tensor("rot", [N, 3], f32, kind="ExternalInput")
    yT = nc.dram_tensor("yT", [OUT, DS], f32, kind="ExternalOutput")

    with TileContext(nc) as tc:
        with (
            tc.tile_pool(name="const", bufs=1) as cp,
            tc.tile_pool(name="xt", bufs=4) as xp,
            tc.tile_pool(name="bups", bufs=3, space="PSUM") as bp,
            tc.tile_pool(name="bu", bufs=2) as up,
            tc.tile_pool(name="tw", bufs=2) as wp,
            tc.tile_pool(name="hh", bufs=1) as hp,
            tc.tile_pool(name="carry", bufs=1) as kp,
            tc.tile_pool(name="yps", bufs=5, space="PSUM") as yp,
            tc.tile_pool(name="ysb", bufs=2) as op_,
        ):
            def load_const(dram, rows, dtype, tagp, eng=None):
                eng = eng or nc.gpsimd
                tiles = []
                for i in range(rows // P):
                    t = cp.tile([P, dram.shape[1]], dtype, tag=f"{tagp}{i}",
                                name=f"{tagp}{i}")
                    eng.dma_start(t[:], dram[i * P:(i + 1) * P, :])
                    tiles.append(t)
                return tiles

            # ---- x + input weights first so PE can start ASAP ----
            btr_t = load_const(btr, IN, f16, "btr", eng=nc.sync)
            xts = []
            for ib in range(IBN):
                xt_t = xp.tile([P, HF], f16, tag="xt", name="xt")
                nc.sync.dma_start(xt_t[:], xT[ib * P:(ib + 1) * P, :])
                xts.append(xt_t)
            bti_t = load_const(bti, IN, f16, "bti", eng=nc.sync)
            cos_t = load_const(cosj, N, f16, "cos")
            sin_t = load_const(sinj, N, f16, "sin")
            xh_t = load_const(xh, HF, f16, "xh")
            vreT_t = load_const(vreT, HF, f16, "vreT")
            vimT_t = load_const(vimT, HF, f16, "vimT")
            bnr_t = load_const(bnr, N, f16, "bnr")
            bni_t = load_const(bni, N, f16, "bni")
            rb_t = load_const(rb, N, f32, "rb")
            rot_t = load_const(rot, N, f32, "rot")

            def input_gemm(nb, pl_tiles, name):
                """Bu^T[n-block, second half] -> fp16 SBUF tile [P, HF]."""
                nsl = slice(nb * P, (nb + 1) * P)
                bu_t = up.tile([P, HF], f16, tag=name, name=name)
                for h in range(HF // HH):
                    t0 = h * HH
                    ps = bp.tile([P, HH], f32, tag="bups", name="bups")
                    for ib in range(IBN):
                        nc.tensor.matmul(
                            ps[:],
                            pl_tiles[ib][:, nsl],
                            xts[ib][:, t0:t0 + HH],
                            start=(ib == 0),
                            stop=(ib == IBN - 1),
                        )
                    nc.scalar.copy(bu_t[:, h * HH:(h + 1) * HH], ps[:])
                return bu_t

            def w_gemm(nb, v_tiles, name):
                """W[n-block, i] = sum_s V[n,s] x[s,i] -> fp16 tile [P, IN]."""
                nsl = slice(nb * P, (nb + 1) * P)
                ps = bp.tile([P, IN], f32, tag="bups", name="bups")
                for sb in range(HF // P):
                    nc.tensor.matmul(
                        ps[:],
                        v_tiles[sb][:, nsl],
                        xh_t[sb][:],
                        start=(sb == 0),
                        stop=(sb == HF // P - 1),
                    )
                w_t = up.tile([P, IN], f16, tag=name, name=name)
                nc.scalar.copy(w_t[:], ps[:])
                return w_t

            # ---- second half Bu GEMMs + twist (starts as soon as x/B land) ----
            e_res, e_ims = [], []
            for nb in range(NB):
                buB_r = input_gemm(nb, btr_t, "buBr")
                buB_i = input_gemm(nb, bti_t, "buBi")
                # twist: e = e^{-i j theta} * Bu
                p1 = wp.tile([P, HF], f16, tag="p1", name="p1")
                p2 = wp.tile([P, HF], f16, tag="p2", name="p2")
                p3 = wp.tile([P, HF], f16, tag="p3", name="p3")
                p4 = wp.tile([P, HF], f16, tag="p4", name="p4")
                e_re = hp.tile([P, HF], f16, tag=f"ere{nb}", name=f"ere{nb}")
                e_im = hp.tile([P, HF], f16, tag=f"eim{nb}", name=f"eim{nb}")
                nc.vector.tensor_tensor(p1[:], cos_t[nb][:], buB_r[:], AOP.mult)
                nc.vector.tensor_tensor(p2[:], sin_t[nb][:], buB_i[:], AOP.mult)
                nc.vector.tensor_tensor(e_re[:], p1[:], p2[:], AOP.add)
                nc.vector.tensor_tensor(p3[:], cos_t[nb][:], buB_i[:], AOP.mult)
                nc.vector.tensor_tensor(p4[:], sin_t[nb][:], buB_r[:], AOP.mult)
                nc.vector.tensor_tensor(e_im[:], p3[:], p4[:], AOP.subtract)
                e_res.append(e_re)
                e_ims.append(e_im)

            # ---- first half: W = V @ x_half (PE), then h_1023 = sum_i B~.W ----
            inits = []
            for nb in range(NB):
                w_re = w_gemm(nb, vreT_t, "wre")
                w_im = w_gemm(nb, vimT_t, "wim")
                dump = wp.tile([P, IN], f16, tag="dump", name="dump")
                s1 = kp.tile([P, 1], f32, tag=f"s1{nb}", name=f"s1{nb}")
                s2 = kp.tile([P, 1], f32, tag=f"s2{nb}", name=f"s2{nb}")
                s3 = kp.tile([P, 1], f32, tag=f"s3{nb}", name=f"s3{nb}")
                s4 = kp.tile([P, 1], f32, tag=f"s4{nb}", name=f"s4{nb}")
                a_re = kp.tile([P, 1], f32, tag=f"are{nb}", name=f"are{nb}")
                a_im = kp.tile([P, 1], f32, tag=f"aim{nb}", name=f"aim{nb}")
                nc.vector.scalar_tensor_tensor(
                    dump[:], bnr_t[nb][:], 1.0, w_re[:], AOP.bypass, AOP.mult,
                    accum_out=s1[:])
                nc.vector.scalar_tensor_tensor(
                    dump[:], bni_t[nb][:], 1.0, w_im[:], AOP.bypass, AOP.mult,
                    accum_out=s2[:])
                nc.vector.scalar_tensor_tensor(
                    dump[:], bnr_t[nb][:], 1.0, w_im[:], AOP.bypass, AOP.mult,
                    accum_out=s3[:])
                nc.vector.scalar_tensor_tensor(
                    dump[:], bni_t[nb][:], 1.0, w_re[:], AOP.bypass, AOP.mult,
                    accum_out=s4[:])
                nc.vector.tensor_tensor(a_re[:], s1[:], s2[:], AOP.subtract)
                nc.vector.tensor_tensor(a_im[:], s3[:], s4[:], AOP.add)
                # carry rotation: init = e^{i theta} * h_1023
                i_re = kp.tile([P, 1], f32, tag=f"ire{nb}", name=f"ire{nb}")
                i_im = kp.tile([P, 1], f32, tag=f"iim{nb}", name=f"iim{nb}")
                u_re = kp.tile([P, 1], f32, tag=f"ure{nb}", name=f"ure{nb}")
                u_im = kp.tile([P, 1], f32, tag=f"uim{nb}", name=f"uim{nb}")
                nc.scalar.mul(u_re[:], a_re[:], rot_t[nb][:, 0:1])
                nc.vector.scalar_tensor_tensor(
                    i_re[:], a_im[:], rot_t[nb][:, 1:2], u_re[:],
                    AOP.mult, AOP.add)
                nc.scalar.mul(u_im[:], a_im[:], rot_t[nb][:, 0:1])
                nc.vector.scalar_tensor_tensor(
                    i_im[:], a_re[:], rot_t[nb][:, 2:3], u_im[:],
                    AOP.mult, AOP.add)
                inits.append((i_re, i_im))

            # ---- scans + untwist ----
            hh_re, hh_im = [], []
            for nb in range(NB):
                i_re, i_im = inits[nb]
                e_re, e_im = e_res[nb], e_ims[nb]
                h_re = wp.tile([P, HF], f16, tag="hre", name="hre")
                h_im = wp.tile([P, HF], f16, tag="him", name="him")
                q1 = wp.tile([P, HF], f16, tag="q1", name="q1")
                q2 = wp.tile([P, HF], f16, tag="q2", name="q2")
                q3 = wp.tile([P, HF], f16, tag="q3", name="q3")
                q4 = wp.tile([P, HF], f16, tag="q4", name="q4")
                hhr = hp.tile([P, HF], f16, tag=f"hhr{nb}", name=f"hhr{nb}")
                hhi = hp.tile([P, HF], f16, tag=f"hhi{nb}", name=f"hhi{nb}")
                for h in range(HF // HH):
                    hs = slice(h * HH, (h + 1) * HH)
                    ir = i_re[:, 0:1] if h == 0 else h_re[:, h * HH - 1:h * HH]
                    ii = i_im[:, 0:1] if h == 0 else h_im[:, h * HH - 1:h * HH]
                    nc.vector.tensor_tensor_scan(
                        h_re[:, hs], rb_t[nb][:, 0:1].broadcast_to((P, HH)), e_re[:, hs], ir,
                        AOP.mult, AOP.add)
                    nc.vector.tensor_tensor_scan(
                        h_im[:, hs], rb_t[nb][:, 0:1].broadcast_to((P, HH)), e_im[:, hs], ii,
                        AOP.mult, AOP.add)
                    nc.vector.tensor_tensor(q1[:, hs], cos_t[nb][:, hs], h_re[:, hs], AOP.mult)
                    nc.vector.tensor_tensor(q2[:, hs], sin_t[nb][:, hs], h_im[:, hs], AOP.mult)
                    nc.vector.tensor_tensor(hhr[:, hs], q1[:, hs], q2[:, hs], AOP.subtract)
                    nc.vector.tensor_tensor(q3[:, hs], cos_t[nb][:, hs], h_im[:, hs], AOP.mult)
                    nc.vector.tensor_tensor(q4[:, hs], sin_t[nb][:, hs], h_re[:, hs], AOP.mult)
                    nc.vector.tensor_tensor(hhi[:, hs], q3[:, hs], q4[:, hs], AOP.add)
                hh_re.append(hhr)
                hh_im.append(hhi)

            # ---- output weights (queue-ordered after input-side DMAs) ----
            ctr_t = load_const(ctr, N, f16, "ctr")
            ctin_t = load_const(ctin, N, f16, "ctin")
            dtw_t = load_const(dtw, IN, f16, "dtw")

            # ---- output GEMMs ----
            for h in range(DS // HH):
                hsl = slice(h * HH, (h + 1) * HH)
                xsl = slice(h * HH, (h + 1) * HH)
                for ob in range(OBN):
                    osl = slice(ob * P, (ob + 1) * P)
                    ps = yp.tile([P, HH], f32, tag="yps", name="yps")
                    nmm = 2 * NB + IBN
                    ops = []
                    for nb in range(NB - 1):
                        ops.append((ctr_t[nb][:, osl], hh_re[nb][:, hsl]))
                        ops.append((ctin_t[nb][:, osl], hh_im[nb][:, hsl]))
                    for ib in range(IBN):
                        ops.append((dtw_t[ib][:, osl], xts[ib][:, xsl]))
                    ops.append((ctr_t[NB - 1][:, osl], hh_re[NB - 1][:, hsl]))
                    ops.append((ctin_t[NB - 1][:, osl], hh_im[NB - 1][:, hsl]))
                    for k, (w, m) in enumerate(ops):
                        nc.tensor.matmul(ps[:], w, m,
                                         start=(k == 0), stop=(k == nmm - 1))
                    ysb = op_.tile([P, HH], f32, tag="ysb", name="ysb")
                    nc.scalar.copy(ysb[:], ps[:])
                    nc.sync.dma_start(yT[osl, hsl], ysb[:])

    nc.compile()
    nc.finalize()
    _CACHE["nc"] = nc
    return nc


def _legalize_waits(nc):
    """This toolchain's walrus accepts only ONE sync-wait per instruction
    (NEURON_ISA_TPB_EVENTS has a single wait slot); Tile's scheduler can emit
    several. Splice wait-carrying NoOps immediately before each offender —
    semantically identical blocking point, one wait per instruction."""
    cnt = 0
    for f in nc.m.functions:
        for bb in f.blocks:
            out = []
            changed = False
            for ins in bb.instructions:
                si = ins.sync_info
                waits = list(si.on_wait) if si and si.on_wait else []
                if len(waits) > 1:
                    changed = True
                    for w in waits[:-1]:
                        nop = mybir.InstNoOp(name=f"waitnop-{cnt}")
                        cnt += 1
                        nop.engine = ins.engine
                        nop.sync_info = mybir.SyncInfo(on_wait=[w], on_update=[])
                        nc.register_instruction(nop)
                        out.append(nop)
                    ins.sync_info = mybir.SyncInfo(
                        on_wait=[waits[-1]], on_update=list(si.on_update or []))
                out.append(ins)
            if changed:
                bb.instructions = out


def _host_prep(x, nu_log, theta_log, gamma_log, B_re, B_im, C_re, C_im, D):
    f64 = np.float64
    nu = np.asarray(nu_log, f64)
    th = np.asarray(theta_log, f64)
    gl = np.asarray(gamma_log, f64)
    r = np.exp(-np.exp(nu))
    theta = np.exp(th)
    gamma = np.exp(gl)

    shared = {
        "btr": np.ascontiguousarray((gamma[:, None] * np.asarray(B_re, f64)).T).astype(np.float16),
        "bti": np.ascontiguousarray((gamma[:, None] * np.asarray(B_im, f64)).T).astype(np.float16),
        "ctr": np.ascontiguousarray(np.asarray(C_re, f64).T).astype(np.float16),
        "ctin": np.ascontiguousarray((-np.asarray(C_im, f64)).T).astype(np.float16),
        "dtw": np.ascontiguousarray(np.asarray(D, f64).T).astype(np.float16),
    }
    j = np.arange(HF, dtype=f64)
    ang = theta[:, None] * j[None, :]
    shared["cosj"] = np.cos(ang).astype(np.float16)
    shared["sinj"] = np.sin(ang).astype(np.float16)
    # V = lam^{1023-s} = r^{1023-s} e^{i (1023-s) theta}, shipped transposed [s, n]
    e = (HF - 1) - j
    mag = np.exp(np.log(r)[:, None] * e[None, :])
    angv = theta[:, None] * e[None, :]
    shared["vreT"] = np.ascontiguousarray((mag * np.cos(angv)).T).astype(np.float16)
    shared["vimT"] = np.ascontiguousarray((mag * np.sin(angv)).T).astype(np.float16)
    shared["bnr"] = (gamma[:, None] * np.asarray(B_re, f64)).astype(np.float16)
    shared["bni"] = (gamma[:, None] * np.asarray(B_im, f64)).astype(np.float16)
    shared["rb"] = np.ascontiguousarray(r[:, None].astype(np.float32))
    shared["rot"] = np.stack(
        [np.cos(theta), -np.sin(theta), np.sin(theta)], axis=1).astype(np.float32)

    x = np.asarray(x, np.float32)
    in_maps = []
    for b in range(BATCH):
        m = dict(shared)
        m["xT"] = np.ascontiguousarray(x[b, HF:].T).astype(np.float16)
        m["xh"] = np.ascontiguousarray(x[b, :HF]).astype(np.float16)
        in_maps.append(m)
    return in_maps


def _run(in_maps, trace=False):
    nc = _build_nc()
    return run_bass_kernel_spmd(nc, in_maps, core_ids=list(range(BATCH)), trace=trace)


def kernel(**inputs):
    in_maps = _host_prep(**inputs)
    res = _run(in_maps, trace=False)
    y = np.stack([np.ascontiguousarray(res.results[b]["yT"].T) for b in range(BATCH)])
    return y.astype(np.float32)


def kernel_traced(**inputs):
    """Like kernel() but returns (y, exec_time_ns). Used by test.py."""
    in_maps = _host_prep(**inputs)
    res = _run(in_maps, trace=True)
    y = np.stack([np.ascontiguousarray(res.results[b]["yT"].T) for b in range(BATCH)])
    return y.astype(np.float32), res.exec_time_ns


# revision 20
# speedup vs baseline: 1.0148x; 1.0105x over previous
"""Trainium2 Bass kernel for CausalDownsamplingLRU.

Algorithm (per core = one batch element; 8 cores, data-parallel over batch):
  With lam = r*e^{i theta} (per state n), h_t = lam*h_{t-1} + Bu_t, and only
  y[:, -DS:] needed:

  1. Input GEMMs (fp16, PE): Bu^T[n,t] = (gamma*B)^T.T @ x^T  (re & im planes)
  2. FIRST half (t < 1024): only h_{1023} is needed (the carry into the
     output window). Computed as a weighted reduction
        h_1023 = sum_s lam^{1023-s} Bu_s
     via tensor_tensor_reduce (elementwise mult + free-dim accumulate),
     4 TTR ops per n-block with chained accumulator init.
  3. SECOND half: phase twist e_j = e^{-i j theta} (.) Bu_{1024+j} decouples
     the complex recurrence into two REAL per-partition scans
        s_j = r*s_{j-1} + e_j   (tensor_tensor_scan, fp32 state),
     with initial s_{-1} = e^{i theta} * h_1023.
  4. Untwist h = e^{+i j theta} (.) s, then output GEMMs:
        y^T = C_re^T.T @ h_re + (-C_im^T).T @ h_im + D^T.T @ x^T
"""
import numpy as np

import concourse.bass as bass
import concourse.bacc as bacc
import concourse.mybir as mybir
from concourse.tile import TileContext
from concourse.bass_utils import run_bass_kernel_spmd

BATCH, T, IN, OUT, N = 8, 2048, 512, 512, 512
DS = 1024
P = 128
NB = N // P    # 4 state blocks
IBN = IN // P  # 4 input blocks
OBN = OUT // P # 4 output blocks
HF = 1024      # half length (= DS)
HH = 512       # matmul moving free-dim (PSUM bank limit for f32 out)

f32 = mybir.dt.float32
f16 = mybir.dt.float16
AOP = mybir.AluOpType

GP_TT = False   # offload untwist partner products to gpsimd

_CACHE = {}


def _build_nc():
    if "nc" in _CACHE:
        return _CACHE["nc"]
    nc = bacc.Bacc()
    xT = nc.dram_tensor("xT", [IN, HF], f16, kind="ExternalInput")  # second half only
    xh = nc.dram_tensor("xh", [HF, IN], f16, kind="ExternalInput")  # first half, untransposed
    btr = nc.dram_tensor("btr", [IN, N], f16, kind="ExternalInput")
    bti = nc.dram_tensor("bti", [IN, N], f16, kind="ExternalInput")
    ctr = nc.dram_tensor("ctr", [N, OUT], f16, kind="ExternalInput")
    ctin = nc.dram_tensor("ctin", [N, OUT], f16, kind="ExternalInput")
    dtw = nc.dram_tensor("dtw", [IN, OUT], f16, kind="ExternalInput")
    vreT = nc.dram_tensor("vreT", [HF, N], f16, kind="ExternalInput")
    vimT = nc.dram_tensor("vimT", [HF, N], f16, kind="ExternalInput")
    bnr = nc.dram_tensor("bnr", [N, IN], f16, kind="ExternalInput")
    bni = nc.dram_tensor("bni", [N, IN], f16, kind="ExternalInput")
    cosj = nc.dram_tensor("cosj", [N, HF], f16, kind="ExternalInput")
    sinj = nc.dram_tensor("sinj", [N, HF], f16, kind="ExternalInput")
    rb = nc.dram_tensor("rb", [N, 1], f32, kind="ExternalInput")
    # rot columns: 0=cos(theta), 1=-sin(theta), 2=sin(theta)
    rot = nc.dram_tensor("rot", [N, 3], f32, kind="ExternalInput")
    yT = nc.dram_tensor("yT", [OUT, DS], f32, kind="ExternalOutput")

    with TileContext(nc) as tc:
        with (
            tc.tile_pool(name="const", bufs=1) as cp,
            tc.tile_pool(name="xt", bufs=4) as xp,
            tc.tile_pool(name="bups", bufs=3, space="PSUM") as bp,
            tc.tile_pool(name="bu", bufs=2) as up,
            tc.tile_pool(name="tw", bufs=2) as wp,
            tc.tile_pool(name="hh", bufs=1) as hp,
            tc.tile_pool(name="carry", bufs=1) as kp,
            tc.tile_pool(name="yps", bufs=5, space="PSUM") as yp,
            tc.tile_pool(name="ysb", bufs=2) as op_,
        ):
            def load_const(dram, rows, dtype, tagp, eng=None):
                eng = eng or nc.gpsimd
                tiles = []
                for i in range(rows // P):
                    t = cp.tile([P, dram.shape[1]], dtype, tag=f"{tagp}{i}",
                                name=f"{tagp}{i}")
                    eng.dma_start(t[:], dram[i * P:(i + 1) * P, :])
                    tiles.append(t)
                return tiles

            # ---- x + input weights first so PE can start ASAP ----
            btr_t = load_const(btr, IN, f16, "btr", eng=nc.sync)
            xts = []
            for ib in range(IBN):
                xt_t = xp.tile([P, HF], f16, tag="xt", name="xt")
                nc.sync.dma_start(xt_t[:, 0:HH], xT[ib * P:(ib + 1) * P, 0:HH])
                xts.append(xt_t)
            bti_t = load_const(bti, IN, f16, "bti", eng=nc.sync)
            for ib in range(IBN):
                nc.sync.dma_start(xts[ib][:, HH:HF], xT[ib * P:(ib + 1) * P, HH:HF])
            cos_t = load_const(cosj, N, f16, "cos")
            sin_t = load_const(sinj, N, f16, "sin")
            xh_t = load_const(xh, HF, f16, "xh")
            vreT_t = load_const(vreT, HF, f16, "vreT")
            vimT_t = load_const(vimT, HF, f16, "vimT")
            bnr_t = load_const(bnr, N, f16, "bnr")
            bni_t = load_const(bni, N, f16, "bni")
            rb_t = load_const(rb, N, f32, "rb")
            rot_t = load_const(rot, N, f32, "rot")

            def input_gemm(nb, pl_tiles, name):
                """Bu^T[n-block, second half] -> fp16 SBUF tile [P, HF]."""
                nsl = slice(nb * P, (nb + 1) * P)
                bu_t = up.tile([P, HF], f16, tag=name, name=name)
                for h in range(HF // HH):
                    t0 = h * HH
                    ps = bp.tile([P, HH], f32, tag="bups", name="bups")
                    for ib in range(IBN):
                        nc.tensor.matmul(
                            ps[:],
                            pl_tiles[ib][:, nsl],
                            xts[ib][:, t0:t0 + HH],
                            start=(ib == 0),
                            stop=(ib == IBN - 1),
                        )
                    nc.scalar.copy(bu_t[:, h * HH:(h + 1) * HH], ps[:])
                return bu_t

            def w_gemm(nb, v_tiles, name):
                """W[n-block, i] = sum_s V[n,s] x[s,i] -> fp16 tile [P, IN]."""
                nsl = slice(nb * P, (nb + 1) * P)
                ps = bp.tile([P, IN], f32, tag="bups", name="bups")
                for sb in range(HF // P):
                    nc.tensor.matmul(
                        ps[:],
                        v_tiles[sb][:, nsl],
                        xh_t[sb][:],
                        start=(sb == 0),
                        stop=(sb == HF // P - 1),
                    )
                w_t = up.tile([P, IN], f16, tag=name, name=name)
                nc.scalar.copy(w_t[:], ps[:])
                return w_t

            # ---- second half Bu GEMMs + twist (starts as soon as x/B land) ----
            e_res, e_ims = [], []
            for nb in range(NB):
                buB_r = input_gemm(nb, btr_t, "buBr")
                buB_i = input_gemm(nb, bti_t, "buBi")
                # twist: e = e^{-i j theta} * Bu
                p1 = wp.tile([P, HF], f16, tag="p1", name="p1")
                p2 = wp.tile([P, HF], f16, tag="p2", name="p2")
                p3 = wp.tile([P, HF], f16, tag="p3", name="p3")
                p4 = wp.tile([P, HF], f16, tag="p4", name="p4")
                e_re = hp.tile([P, HF], f16, tag=f"ere{nb}", name=f"ere{nb}")
                e_im = hp.tile([P, HF], f16, tag=f"eim{nb}", name=f"eim{nb}")
                nc.vector.tensor_tensor(p1[:], cos_t[nb][:], buB_r[:], AOP.mult)
                nc.vector.tensor_tensor(p2[:], sin_t[nb][:], buB_i[:], AOP.mult)
                nc.vector.tensor_tensor(e_re[:], p1[:], p2[:], AOP.add)
                nc.vector.tensor_tensor(p3[:], cos_t[nb][:], buB_i[:], AOP.mult)
                nc.vector.tensor_tensor(p4[:], sin_t[nb][:], buB_r[:], AOP.mult)
                nc.vector.tensor_tensor(e_im[:], p3[:], p4[:], AOP.subtract)
                e_res.append(e_re)
                e_ims.append(e_im)

            # ---- first half: W = V @ x_half (PE), then h_1023 = sum_i B~.W ----
            inits = []
            for nb in range(NB):
                w_re = w_gemm(nb, vreT_t, "wre")
                w_im = w_gemm(nb, vimT_t, "wim")
                dump = wp.tile([P, IN], f16, tag="dump", name="dump")
                s1 = kp.tile([P, 1], f32, tag=f"s1{nb}", name=f"s1{nb}")
                s2 = kp.tile([P, 1], f32, tag=f"s2{nb}", name=f"s2{nb}")
                s3 = kp.tile([P, 1], f32, tag=f"s3{nb}", name=f"s3{nb}")
                s4 = kp.tile([P, 1], f32, tag=f"s4{nb}", name=f"s4{nb}")
                a_re = kp.tile([P, 1], f32, tag=f"are{nb}", name=f"are{nb}")
                a_im = kp.tile([P, 1], f32, tag=f"aim{nb}", name=f"aim{nb}")
                nc.vector.scalar_tensor_tensor(
                    dump[:], bnr_t[nb][:], 1.0, w_re[:], AOP.bypass, AOP.mult,
                    accum_out=s1[:])
                nc.vector.scalar_tensor_tensor(
                    dump[:], bni_t[nb][:], 1.0, w_im[:], AOP.bypass, AOP.mult,
                    accum_out=s2[:])
                nc.vector.scalar_tensor_tensor(
                    dump[:], bnr_t[nb][:], 1.0, w_im[:], AOP.bypass, AOP.mult,
                    accum_out=s3[:])
                nc.vector.scalar_tensor_tensor(
                    dump[:], bni_t[nb][:], 1.0, w_re[:], AOP.bypass, AOP.mult,
                    accum_out=s4[:])
                nc.vector.tensor_tensor(a_re[:], s1[:], s2[:], AOP.subtract)
                nc.vector.tensor_tensor(a_im[:], s3[:], s4[:], AOP.add)
                # carry rotation: init = e^{i theta} * h_1023
                i_re = kp.tile([P, 1], f32, tag=f"ire{nb}", name=f"ire{nb}")
                i_im = kp.tile([P, 1], f32, tag=f"iim{nb}", name=f"iim{nb}")
                u_re = kp.tile([P, 1], f32, tag=f"ure{nb}", name=f"ure{nb}")
                u_im = kp.tile([P, 1], f32, tag=f"uim{nb}", name=f"uim{nb}")
                nc.scalar.mul(u_re[:], a_re[:], rot_t[nb][:, 0:1])
                nc.vector.scalar_tensor_tensor(
                    i_re[:], a_im[:], rot_t[nb][:, 1:2], u_re[:],
                    AOP.mult, AOP.add)
                nc.scalar.mul(u_im[:], a_im[:], rot_t[nb][:, 0:1])
                nc.vector.scalar_tensor_tensor(
                    i_im[:], a_re[:], rot_t[nb][:, 2:3], u_im[:],
                    AOP.mult, AOP.add)
                inits.append((i_re, i_im))

            # ---- scans + untwist ----
            hh_re, hh_im = [], []
            for nb in range(NB):
                i_re, i_im = inits[nb]
                e_re, e_im = e_res[nb], e_ims[nb]
                h_re = wp.tile([P, HF], f16, tag="hre", name="hre")
                h_im = wp.tile([P, HF], f16, tag="him", name="him")
                q1 = wp.tile([P, HF], f16, tag="q1", name="q1")
                q2 = wp.tile([P, HF], f16, tag="q2", name="q2")
                q3 = wp.tile([P, HF], f16, tag="q3", name="q3")
                q4 = wp.tile([P, HF], f16, tag="q4", name="q4")
                hhr = hp.tile([P, HF], f16, tag=f"hhr{nb}", name=f"hhr{nb}")
                hhi = hp.tile([P, HF], f16, tag=f"hhi{nb}", name=f"hhi{nb}")
                for h in range(HF // HH):
                    hs = slice(h * HH, (h + 1) * HH)
                    ir = i_re[:, 0:1] if h == 0 else h_re[:, h * HH - 1:h * HH]
                    ii = i_im[:, 0:1] if h == 0 else h_im[:, h * HH - 1:h * HH]
                    nc.vector.tensor_tensor_scan(
                        h_re[:, hs], rb_t[nb][:, 0:1].broadcast_to((P, HH)), e_re[:, hs], ir,
                        AOP.mult, AOP.add)
                    nc.vector.tensor_tensor_scan(
                        h_im[:, hs], rb_t[nb][:, 0:1].broadcast_to((P, HH)), e_im[:, hs], ii,
                        AOP.mult, AOP.add)
                    nc.vector.tensor_tensor(q1[:, hs], cos_t[nb][:, hs], h_re[:, hs], AOP.mult)
                    nc.vector.tensor_tensor(q2[:, hs], sin_t[nb][:, hs], h_im[:, hs], AOP.mult)
                    nc.vector.tensor_tensor(hhr[:, hs], q1[:, hs], q2[:, hs], AOP.subtract)
                    nc.vector.tensor_tensor(q3[:, hs], cos_t[nb][:, hs], h_im[:, hs], AOP.mult)
                    nc.vector.tensor_tensor(q4[:, hs], sin_t[nb][:, hs], h_re[:, hs], AOP.mult)
                    nc.vector.tensor_tensor(hhi[:, hs], q3[:, hs], q4[:, hs], AOP.add)
                hh_re.append(hhr)
                hh_im.append(hhi)

            # ---- output weights (queue-ordered after input-side DMAs) ----
            ctr_t = load_const(ctr, N, f16, "ctr")
            ctin_t = load_const(ctin, N, f16, "ctin")
            dtw_t = load_const(dtw, IN, f16, "dtw")

            # ---- output GEMMs ----
            for h in range(DS // HH):
                hsl = slice(h * HH, (h + 1) * HH)
                xsl = slice(h * HH, (h + 1) * HH)
                for ob in range(OBN):
                    osl = slice(ob * P, (ob + 1) * P)
                    gi = h * OBN + ob
                    pool = yp if gi < 5 else bp
                    ps = pool.tile([P, HH], f32, tag="bups" if gi >= 5 else "yps",
                                   name="yps")
                    nmm = 2 * NB + IBN
                    ops = []
                    for nb in range(NB - 1):
                        ops.append((ctr_t[nb][:, osl], hh_re[nb][:, hsl]))
                        ops.append((ctin_t[nb][:, osl], hh_im[nb][:, hsl]))
                    for ib in range(IBN):
                        ops.append((dtw_t[ib][:, osl], xts[ib][:, xsl]))
                    ops.append((ctr_t[NB - 1][:, osl], hh_re[NB - 1][:, hsl]))
                    ops.append((ctin_t[NB - 1][:, osl], hh_im[NB - 1][:, hsl]))
                    for k, (w, m) in enumerate(ops):
                        nc.tensor.matmul(ps[:], w, m,
                                         start=(k == 0), stop=(k == nmm - 1))
                    ysb = op_.tile([P, HH], f32, tag="ysb", name="ysb")
                    nc.scalar.copy(ysb[:], ps[:])
                    nc.sync.dma_start(yT[osl, hsl], ysb[:])

    nc.compile()
    nc.finalize()
    _CACHE["nc"] = nc
    return nc


def _legalize_waits(nc):
    """This toolchain's walrus accepts only ONE sync-wait per instruction
    (NEURON_ISA_TPB_EVENTS has a single wait slot); Tile's scheduler can emit
    several. Splice wait-carrying NoOps immediately before each offender —
    semantically identical blocking point, one wait per instruction."""
    cnt = 0
    for f in nc.m.functions:
        for bb in f.blocks:
            out = []
            changed = False
            for ins in bb.instructions:
                si = ins.sync_info
                waits = list(si.on_wait) if si and si.on_wait else []
                if len(waits) > 1:
                    changed = True
                    for w in waits[:-1]:
                        nop = mybir.InstNoOp(name=f"waitnop-{cnt}")
                        cnt += 1
                        nop.engine = ins.engine
                        nop.sync_info = mybir.SyncInfo(on_wait=[w], on_update=[])
                        nc.register_instruction(nop)
                        out.append(nop)
                    ins.sync_info = mybir.SyncInfo(
                        on_wait=[waits[-1]], on_update=list(si.on_update or []))
                out.append(ins)
            if changed:
                bb.instructions = out


def _host_prep(x, nu_log, theta_log, gamma_log, B_re, B_im, C_re, C_im, D):
    f64 = np.float64
    nu = np.asarray(nu_log, f64)
    th = np.asarray(theta_log, f64)
    gl = np.asarray(gamma_log, f64)
    r = np.exp(-np.exp(nu))
    theta = np.exp(th)
    gamma = np.exp(gl)

    shared = {
        "btr": np.ascontiguousarray((gamma[:, None] * np.asarray(B_re, f64)).T).astype(np.float16),
        "bti": np.ascontiguousarray((gamma[:, None] * np.asarray(B_im, f64)).T).astype(np.float16),
        "ctr": np.ascontiguousarray(np.asarray(C_re, f64).T).astype(np.float16),
        "ctin": np.ascontiguousarray((-np.asarray(C_im, f64)).T).astype(np.float16),
        "dtw": np.ascontiguousarray(np.asarray(D, f64).T).astype(np.float16),
    }
    j = np.arange(HF, dtype=f64)
    ang = theta[:, None] * j[None, :]
    shared["cosj"] = np.cos(ang).astype(np.float16)
    shared["sinj"] = np.sin(ang).astype(np.float16)
    # V = lam^{1023-s} = r^{1023-s} e^{i (1023-s) theta}, shipped transposed [s, n]
    e = (HF - 1) - j
    mag = np.exp(np.log(r)[:, None] * e[None, :])
    angv = theta[:, None] * e[None, :]
    shared["vreT"] = np.ascontiguousarray((mag * np.cos(angv)).T).astype(np.float16)
    shared["vimT"] = np.ascontiguousarray((mag * np.sin(angv)).T).astype(np.float16)
    shared["bnr"] = (gamma[:, None] * np.asarray(B_re, f64)).astype(np.float16)
    shared["bni"] = (gamma[:, None] * np.asarray(B_im, f64)).astype(np.float16)
    shared["rb"] = np.ascontiguousarray(r[:, None].astype(np.float32))
    shared["rot"] = np.stack(
        [np.cos(theta), -np.sin(theta), np.sin(theta)], axis=1).astype(np.float32)

    x = np.asarray(x, np.float32)
    in_maps = []
    for b in range(BATCH):
        m = dict(shared)
        m["xT"] = np.ascontiguousarray(x[b, HF:].T).astype(np.float16)
        m["xh"] = np.ascontiguousarray(x[b, :HF]).astype(np.float16)
        in_maps.append(m)
    return in_maps


def _run(in_maps, trace=False):
    nc = _build_nc()
    return run_bass_kernel_spmd(nc, in_maps, core_ids=list(range(BATCH)), trace=trace)


def kernel(**inputs):
    in_maps = _host_prep(**inputs)
    res = _run(in_maps, trace=False)
    y = np.stack([np.ascontiguousarray(res.results[b]["yT"].T) for b in range(BATCH)])
    return y.astype(np.float32)


def kernel_traced(**inputs):
    """Like kernel() but returns (y, exec_time_ns). Used by test.py."""
    in_maps = _host_prep(**inputs)
    res = _run(in_maps, trace=True)
    y = np.stack([np.ascontiguousarray(res.results[b]["yT"].T) for b in range(BATCH)])
    return y.astype(np.float32), res.exec_time_ns
